# revision 17
# baseline (speedup 1.0000x reference)
"""nn_AlignerOT distributed Trainium2 kernel, v3 (8 NeuronCores).

Per-token 1D entropic OT: ITERS log-domain Sinkhorn iterations over per-token
[512,512] cost matrices cost = 300*(x_i - y_j)^2, then ot = mean_n(P)*D*SCALE
+ delta_ot and out = src @ ot.

v3 changes vs v2:
- Software-pipelined halves: the 64-matvec g-update burst of half h is
  interleaved (4-matvec chunks) into the PE stream of half h+1's f-phase,
  and g_tail/f_smalls overlap the next half's compute. PE stays dense
  (p-state ramp) and ACT no longer idles during the burst.
- Banding width parameterized (MARGIN); U assembly generated from segment
  cover plan.
- Row sums fully on DVE/Pool (steady iterations do a single full-width exp
  per token, no ACT accumulate path); pair-tree adds and small copies are
  split between DVE and the otherwise idle Pool engine.
- Aitken extrapolation: the final iteration's f/g updates are scaled by
  (1+BOOST_C), approximating the reference's extra Sinkhorn iterations at
  zero cost. Validated offline against the fp32 reference.
"""

import sys

sys.path.insert(0, "/opt/trn_rl_repo")

import numpy as np
import ml_dtypes

from concourse import bacc, tile, mybir
from concourse import hw_specs
from concourse.bass_utils import run_bass_kernel_spmd

F32 = mybir.dt.float32
BF16 = mybir.dt.bfloat16

REG = 0.1
SCALE = 300.0
D = 512
NCORES = 8
NTOK = 32            # tokens per core
NTOT = NCORES * NTOK
ITERS = 50           # matched Sinkhorn iterations (may be lowered w/ boost)
BOOST_C = 0.0        # Aitken boost on the last iteration's updates
MARGIN = 48          # band half-margin per 128-row tile
USE_POOL = True      # route tensor_tensor side ops to the Pool engine
SL = 256             # psum slot stride per window (bank-aligned)
RLA = float(REG * np.log(1.0 / D))
LA = float(np.log(1.0 / D))


def _band(margin):
    W = 128 + 2 * margin
    LO = [max(0, min(D - W, 128 * t - margin)) for t in range(4)]
    bpts = sorted(set([0, D] + LO + [l + W for l in LO]))
    segs = [(a, b, [t for t in range(4) if LO[t] <= a and b <= LO[t] + W])
            for a, b in zip(bpts[:-1], bpts[1:])]
    # coalesced copy/add plan for the U assembly
    plan_copy, plan_add = [], []
    for a, b, ts in segs:
        t0 = ts[0]
        if plan_copy and plan_copy[-1][2] == t0 and plan_copy[-1][1] == a:
            plan_copy[-1] = (plan_copy[-1][0], b, t0)
        else:
            plan_copy.append((a, b, t0))
        for t in ts[1:]:
            if plan_add and plan_add[-1][2] == t and plan_add[-1][1] == a:
                plan_add[-1] = (plan_add[-1][0], b, t)
            else:
                plan_add.append((a, b, t))
    return W, LO, plan_copy, plan_add


# Force every activation onto the one table set holding Exp and Ln (v1 trick).
_orig_get_tables = hw_specs.get_activation_tables


def _patched_tables(arch):
    t = _orig_get_tables(arch)
    keep = "natural_log_exp_and_others"
    if keep in t:
        t = {k: (v if k == keep else set()) for k, v in t.items()}
    return t


hw_specs.get_activation_tables = _patched_tables
bacc.get_activation_tables = _patched_tables


def _limbs3(a):
    a = np.asarray(a, np.float32)
    l0 = a.astype(ml_dtypes.bfloat16)
    r1 = a - l0.astype(np.float32)
    l1 = r1.astype(ml_dtypes.bfloat16)
    r2 = r1 - l1.astype(np.float32)
    l2 = r2.astype(ml_dtypes.bfloat16)
    return l0, l1, l2


def _lhsT_host(v):
    """[NTOK,512] f32 -> [12,16384] bf16 rows [1,1,1,v0,v0,v0,v1,v1,v2,0,0,0]."""
    v0, v1, v2 = _limbs3(v.reshape(-1))
    ones = np.ones(NTOK * 512, ml_dtypes.bfloat16)
    zero = np.zeros(NTOK * 512, ml_dtypes.bfloat16)
    return np.stack([ones, ones, ones, v0, v0, v0, v1, v1, v2, zero, zero, zero])


def _rhs_host(alpha, beta):
    """[12,16384] bf16 rows [a0,a1,a2,b0,b1,b2,b0,b1,b0,-1,-1,-1]."""
    a0, a1, a2 = _limbs3(alpha.reshape(-1))
    b0, b1, b2 = _limbs3(beta.reshape(-1))
    mone = np.full(NTOK * 512, -1.0, ml_dtypes.bfloat16)
    return np.stack([a0, a1, a2, b0, b1, b2, b0, b1, b0, mone, mone, mone])


def _build(iters=ITERS, margin=MARGIN, boost_c=BOOST_C):
    W, LO, PLAN_COPY, PLAN_ADD = _band(margin)
    DW = W
    nc = bacc.Bacc("TRN2", target_bir_lowering=False, debug=False, num_devices=NCORES)

    lhsT1_e = nc.dram_tensor("lhsT1", [24, 8192], BF16, kind="ExternalInput")
    lhsT2_e = nc.dram_tensor("lhsT2", [24, 8192], BF16, kind="ExternalInput")
    rhs1_e = nc.dram_tensor("rhs1i", [24, 8192], BF16, kind="ExternalInput")
    rhs2_e = nc.dram_tensor("rhs2i", [24, 8192], BF16, kind="ExternalInput")
    lhsT1o_e = nc.dram_tensor("lhsT1o", [24, 8192], BF16, kind="ExternalInput")
    rhs1o_e = nc.dram_tensor("rhs1o", [24, 8192], BF16, kind="ExternalInput")
    permx_e = nc.dram_tensor("permx", [NTOK * 4 * 128, D], BF16, kind="ExternalInput")
    permy_e = nc.dram_tensor("permy", [NTOK * 4 * 128, D], BF16, kind="ExternalInput")
    xT_e = nc.dram_tensor("xT", [D, NTOK], F32, kind="ExternalInput")
    delta_e = nc.dram_tensor("delta", [D, D], F32, kind="ExternalInput")
    out_e = nc.dram_tensor("out", [NTOK, D], F32, kind="ExternalOutput")

    pool_eng = (lambda: nc.gpsimd) if USE_POOL else (lambda: nc.vector)
    with tile.TileContext(nc, num_cores=NCORES) as tc:
        with (
            tc.tile_pool(name="state", bufs=1) as st,
            tc.tile_pool(name="work", bufs=2) as wk,
            tc.tile_pool(name="dumps", bufs=32) as dp,
            tc.tile_pool(name="psum", bufs=3, space="PSUM") as ps,
            tc.tile_pool(name="psumU", bufs=1, space="PSUM") as psU,
            tc.tile_pool(name="dram", bufs=1, space="DRAM") as dr,
        ):
            lhsT = [st.tile([128, 8192], BF16, name=f"lhsT{p}") for p in range(2)]
            rhs = [st.tile([128, 8192], BF16, name=f"rhs{p}") for p in range(2)]
            sig = [st.tile([128, 128], F32, name=f"sig{p}") for p in range(2)]
            sigu = st.tile([128, 128], F32)
            biasc = st.tile([128, 128], F32)
            Scol = [st.tile([128, 128], F32, name=f"Scol{p}") for p in range(2)]
            acol = st.tile([128, 128], BF16)
            az = st.tile([128, 2048], BF16)
            alpha_sb = [st.tile([16, 512], F32, name=f"alpha{h}") for h in range(2)]
            Lcat = [st.tile([128, 408], BF16, name=f"Lcat{p}") for p in range(2)]
            DS = [[st.tile([128, 4 * DW], BF16, name=f"ds{h}_{j}")
                   for j in range(16)] for h in range(2)]
            Pacc = st.tile([128, 4 * D], F32)
            srcT = st.tile([128, 4 * NTOK], F32)
            out_sb = st.tile([NTOK, D], F32)

            for gg in range(2):
                nc.sync.dma_start(out=lhsT[0][gg * 64 : gg * 64 + 12, :],
                                  in_=lhsT1_e.ap()[gg * 12 : (gg + 1) * 12, :])
                nc.sync.dma_start(out=lhsT[1][gg * 64 : gg * 64 + 12, :],
                                  in_=lhsT2_e.ap()[gg * 12 : (gg + 1) * 12, :])
                nc.sync.dma_start(out=rhs[0][gg * 64 : gg * 64 + 12, :],
                                  in_=rhs1_e.ap()[gg * 12 : (gg + 1) * 12, :])
                nc.sync.dma_start(out=rhs[1][gg * 64 : gg * 64 + 12, :],
                                  in_=rhs2_e.ap()[gg * 12 : (gg + 1) * 12, :])
            for t in range(4):
                nc.sync.dma_start(out=srcT[:, t * NTOK : (t + 1) * NTOK],
                                  in_=xT_e.ap()[t * 128 : (t + 1) * 128, :])
            la_bias = st.tile([128, 1], F32)
            nc.vector.memset(la_bias[:], LA)
            nc.vector.memset(Pacc[:], 0.0)
            nc.vector.memset(sig[0][:], 0.0)
            nc.vector.memset(sig[1][:], 0.0)
            nc.vector.memset(az[:], 0.0)

            def f_token(n, fresh, p=0):
                """Banded matmuls + exp (packed dump) + row sums for token n.
                Steady path: one full-width exp, pair-tree add (DVE/Pool
                alternating) + DVE reduce. Fresh path: DVE max + 4 biased
                exps with ACT accumulation (as v2)."""
                pt = ps.tile([128, 1024], F32, tag="mm", name="pt")
                gg, bb = n % 2, n // 2
                for t in range(4):
                    nc.tensor.matmul(
                        pt[:, t * SL : t * SL + W],
                        lhsT[p][gg * 64 : gg * 64 + 12,
                                bb * 512 + t * 128 : bb * 512 + (t + 1) * 128],
                        rhs[p][gg * 64 : gg * 64 + 12,
                               bb * 512 + LO[t] : bb * 512 + LO[t] + W],
                        start=True, stop=True)
                dump = DS[(n // 16) % 2][n % 16]
                if fresh:
                    nc.vector.tensor_reduce(
                        sigu[:, n * 4 : (n + 1) * 4],
                        pt[:].rearrange("p (t f) -> p t f", t=4)[:, :, 0:W],
                        axis=mybir.AxisListType.X, op=mybir.AluOpType.max)
                    nc.vector.tensor_scalar(
                        out=biasc[:, n * 4 : (n + 1) * 4],
                        in0=sigu[:, n * 4 : (n + 1) * 4],
                        scalar1=-1.0 / REG, scalar2=None,
                        op0=mybir.AluOpType.mult)
                    for t in range(4):
                        col = n * 4 + t
                        nc.scalar.activation(
                            dump[:, t * DW : t * DW + W],
                            pt[:, t * SL : t * SL + W],
                            mybir.ActivationFunctionType.Exp,
                            bias=biasc[:, col : col + 1], scale=1.0 / REG,
                            accum_out=Scol[p][:, col : col + 1])
                else:
                    nc.scalar.activation(
                        dump[:].rearrange("p (t f) -> p t f", t=4),
                        pt[:].rearrange("p (t f) -> p t f", t=4)[:, :, 0:W],
                        mybir.ActivationFunctionType.Exp,
                        scale=1.0 / REG)
                    ptree = wk.tile([128, 4 * (W // 2)], BF16, tag="ptree",
                                    name="ptree", bufs=2)
                    eng = pool_eng() if (n % 2 == 0) else nc.vector
                    with nc.allow_low_precision(reason="bf16 pair-tree level 1"):
                        eng.tensor_tensor(
                            ptree[:].rearrange("p (t f) -> p t f", t=4),
                            dump[:].rearrange("p (t f) -> p t f", t=4)[:, :, 0 : W // 2],
                            dump[:].rearrange("p (t f) -> p t f", t=4)[:, :, W // 2 : W],
                            mybir.AluOpType.add)
                    nc.vector.tensor_reduce(
                        Scol[p][:, n * 4 : (n + 1) * 4],
                        ptree[:].rearrange("p (t f) -> p t f", t=4),
                        axis=mybir.AxisListType.X, op=mybir.AluOpType.add)

            def f_smalls(half, fresh, capture=False, p=0, write_sig_limbs=True,
                         alpha_dst=None, assemble_alpha=False, boost=1.0):
                """sigma' = sigma + [fresh max] + boost*reg*ln(S); write sigma
                limbs into lhsT[p] rows 9-11 (col-major via DMA transpose)."""
                c0, c1 = half * 64, (half + 1) * 64
                f0 = half * 8192
                sg = sig[p][:, c0:c1]
                lnS = wk.tile([128, 64], F32, tag="lnS", name="lnS")
                nc.scalar.activation(lnS[:], Scol[p][:, c0:c1], mybir.ActivationFunctionType.Ln)
                if fresh:
                    tmp = wk.tile([128, 64], F32, tag="tmp", name="tmp")
                    nc.vector.scalar_tensor_tensor(
                        out=tmp[:], in0=lnS[:], scalar=REG * boost, in1=sigu[:, c0:c1],
                        op0=mybir.AluOpType.mult, op1=mybir.AluOpType.add)
                    nc.vector.tensor_tensor(sg, tmp[:], sg, mybir.AluOpType.add)
                else:
                    nc.vector.scalar_tensor_tensor(
                        out=sg, in0=lnS[:], scalar=REG * boost, in1=sg,
                        op0=mybir.AluOpType.mult, op1=mybir.AluOpType.add)
                srcs = []
                if write_sig_limbs:
                    srcs.append((sg, lhsT[p], 9, capture))
                if alpha_dst is not None:
                    acm = wk.tile([128, 64], F32, tag="acm", name="acm")
                    nc.vector.tensor_scalar(
                        out=acm[:], in0=sg, scalar1=-1.0, scalar2=RLA,
                        op0=mybir.AluOpType.mult, op1=mybir.AluOpType.add)
                    srcs.append((acm[:], alpha_dst, 0, False))
                for src_cm, dst, base, cap in srcs:
                    # permuted column layout: col c0 + g*32 + b*4 + t holds
                    # token j = 2b+g, tile t (so each partition group g flattens
                    # to one contiguous operand row segment)
                    sgp = wk.tile([128, 64], F32, tag="sgp", name="sgp")
                    nc.vector.tensor_copy(
                        sgp[:].rearrange("p (g b t) -> p g b t", g=2, b=8),
                        src_cm.rearrange("p (b g t) -> p g b t", b=8, g=2))
                    L0 = wk.tile([128, 128], BF16, tag="L0", name="L0")
                    L1 = wk.tile([128, 128], BF16, tag="L1", name="L1")
                    L2 = wk.tile([128, 128], BF16, tag="L2", name="L2")
                    R1 = wk.tile([128, 64], F32, tag="R1", name="R1")
                    R2 = wk.tile([128, 64], F32, tag="R2", name="R2")
                    with nc.allow_low_precision(reason="bf16 limb split"):
                        nc.vector.tensor_copy(L0[:, c0:c1], sgp[:])
                        pool_eng().tensor_tensor(R1[:], sgp[:], L0[:, c0:c1], mybir.AluOpType.subtract)
                        nc.vector.tensor_copy(L1[:, c0:c1], R1[:])
                        pool_eng().tensor_tensor(R2[:], R1[:], L1[:, c0:c1], mybir.AluOpType.subtract)
                        nc.vector.tensor_copy(L2[:, c0:c1], R2[:])
                    AT = None
                    if base == 0 and assemble_alpha:
                        AT = [wk.tile([16, 512], BF16, tag=f"AT{l}", name=f"AT{l}", bufs=1)
                              for l in range(3)]
                    for k, L in enumerate((L0, L1, L2)):
                        LT = wk.tile([128, 128], BF16, tag=f"LT{k}", name=f"LT{k}")
                        nc.sync.dma_start(out=LT[:], in_=L[:], transpose=True)
                        for g in range(2):
                            nc.sync.dma_start(
                                out=dst[base + k + g * 64 : base + k + g * 64 + 1,
                                        half * 4096 : (half + 1) * 4096],
                                in_=LT[c0 + g * 32 : c0 + (g + 1) * 32, :])
                        if cap:
                            for g in range(2):
                                s = 3 * 64 * half + 12 * g + k
                                nc.vector.tensor_copy(
                                    Lcat[0][:, s : s + 24 * 8].rearrange(
                                        "p (b q) -> p b q", b=8)[:, :, 0:12:3],
                                    L[:, c0 + g * 32 : c0 + (g + 1) * 32].rearrange(
                                        "p (b t) -> p b t", b=8))
                        if AT is not None:
                            for t in range(4):
                                for g in range(2):
                                    nc.sync.dma_start(
                                        out=AT[k][g * 8 : (g + 1) * 8,
                                                  t * 128 : (t + 1) * 128],
                                        in_=LT[c0 + g * 32 + t : c0 + (g + 1) * 32 : 4, :])
                    if AT is not None:
                        tmp2 = wk.tile([16, 512], F32, tag="tmp2", name="tmp2", bufs=1)
                        nc.vector.tensor_tensor(tmp2[:], AT[0][:], AT[1][:], mybir.AluOpType.add)
                        nc.vector.tensor_tensor(alpha_sb[half][:], tmp2[:], AT[2][:], mybir.AluOpType.add)

            def recip_az(half, part):
                """alpha=1/S and az one-hot columns for this half's az bank.
                az position for source token nl, window t is
                (4*nl+t)*16 + rho(nl) with rho(nl) = (nl%2)*8 + nl//2 (slot
                rows permuted parity-major so downstream DMAs flatten)."""
                c0 = half * 64
                a0 = half * 1024
                lo, hi = (0, 48) if part == 0 else (48, 64)
                with nc.allow_low_precision(reason="alpha bf16 feeds bf16 matvec"):
                    nc.vector.reciprocal(acol[:, c0 + lo : c0 + hi],
                                         Scol[0][:, c0 + lo : c0 + hi])
                nlo, cnt = (0, 12) if part == 0 else (12, 4)
                for par in range(2):
                    ms = [m for m in range(8) if nlo <= 2 * m + par < nlo + cnt]
                    m0, mc = ms[0], len(ms)
                    for t in range(4):
                        d0 = a0 + 16 * t + 72 * par + 129 * m0
                        s0 = c0 + (2 * m0 + par) * 4 + t
                        nc.vector.tensor_copy(
                            az[:, d0 : d0 + 129 * (mc - 1) + 1 : 129],
                            acol[:, s0 : s0 + 8 * (mc - 1) + 1 : 8])

            def mv_chunk(ctx, nl, pair):
                """2 banded matvecs (windows `pair`, different psum banks) for
                source token nl of the previous half. Each bank keeps exactly
                one open accumulation group: pair (0,2) runs nl=0..15 first,
                then pair (1,3) -- same-bank groups never interleave."""
                half = ctx["half"]
                slots = ctx["slots"]
                snap = ctx["snap"]
                a0 = half * 1024
                for t in pair:
                    dst = slots[t // 2][0:16, (t % 2) * SL : (t % 2) * SL + W]
                    nc.tensor.matmul(
                        dst,
                        az[:, a0 + (4 * nl + t) * 16 : a0 + (4 * nl + t) * 16 + 16],
                        snap[nl][:, t * DW : t * DW + W],
                        start=(nl == 0), stop=(nl == 15))

            def g_tail(ctx):
                """Assemble U from the shifted slots, then ln + alpha/rhs
                update (batched over the half's 16 tokens)."""
                half = ctx["half"]
                slots = ctx["slots"]
                capture = ctx["capture"]
                boost = ctx["boost"]

                def slot_ap(t, a, b):
                    return slots[t // 2][0:16, (t % 2) * SL + a - LO[t] : (t % 2) * SL + b - LO[t]]

                Usb = wk.tile([16, 512], F32, tag="Usb", name="Usb", bufs=1)
                for a, b, t in PLAN_COPY:
                    nc.vector.tensor_copy(Usb[:, a:b], slot_ap(t, a, b))
                for a, b, t in PLAN_ADD:
                    nc.vector.tensor_tensor(Usb[:, a:b], Usb[:, a:b],
                                            slot_ap(t, a, b), mybir.AluOpType.add)
                lnu = wk.tile([16, 512], F32, tag="lnu", name="lnu", bufs=1)
                nc.scalar.activation(lnu[:], Usb[:], mybir.ActivationFunctionType.Ln)
                nc.vector.scalar_tensor_tensor(
                    out=alpha_sb[half][:], in0=lnu[:], scalar=-REG * boost,
                    in1=alpha_sb[half][:],
                    op0=mybir.AluOpType.mult, op1=mybir.AluOpType.add)
                Lg0 = wk.tile([16, 512], BF16, tag="Lg0", name="Lg0", bufs=1)
                Lg1 = wk.tile([16, 512], BF16, tag="Lg1", name="Lg1", bufs=1)
                Lg2 = wk.tile([16, 512], BF16, tag="Lg2", name="Lg2", bufs=1)
                Rg1 = wk.tile([16, 512], F32, tag="Rg1", name="Rg1", bufs=1)
                with nc.allow_low_precision(reason="bf16 limb split"):
                    nc.vector.tensor_copy(Lg0[:], alpha_sb[half][:])
                    pool_eng().tensor_tensor(Rg1[:], alpha_sb[half][:], Lg0[:], mybir.AluOpType.subtract)
                    nc.vector.tensor_copy(Lg1[:], Rg1[:])
                    pool_eng().tensor_tensor(Lg2[:], Rg1[:], Lg1[:], mybir.AluOpType.subtract)
                for l, Lg in enumerate((Lg0, Lg1, Lg2)):
                    # alpha_sb rows are parity-major: row r holds token
                    # 2*(r%8) + r//8, so each 8-row group flattens into one
                    # contiguous operand row segment per partition group
                    for g in range(2):
                        nc.sync.dma_start(
                            out=rhs[0][l + g * 64 : l + g * 64 + 1,
                                       half * 4096 : (half + 1) * 4096],
                            in_=Lg[g * 8 : (g + 1) * 8, :])
                    if capture:
                        for t in range(4):
                            TT = wk.tile([128, 16], BF16, tag="TT", name="TT", bufs=4)
                            nc.sync.dma_start(out=TT[:],
                                              in_=Lg[:, t * 128 : (t + 1) * 128],
                                              transpose=True)
                            for g in range(2):
                                s = 3 * (64 * half + t) + l + 12 * g
                                nc.vector.tensor_copy(
                                    Lcat[1][:, s : s + 24 * 7 + 1 : 24],
                                    TT[:, g * 8 : (g + 1) * 8])

            def steady_half(h, hp_ctx, fresh=False, capture=False, boost=1.0):
                """One pipelined half: 16 f_tokens with the previous half's
                matvec chunks interleaved, g_tail of the previous half at
                slot 10, own f_smalls at the end. Returns this half's ctx."""
                snap = DS[h]
                for j in range(16):
                    f_token(h * 16 + j, fresh)
                    if j == 11:
                        recip_az(h, 0)
                    if hp_ctx is not None:
                        if j == 0:
                            hp_ctx["slots"] = [
                                psU.tile([16, 512], F32, tag=f"slotp{i}",
                                         name=f"slotp{i}") for i in range(2)]
                        pair = (0, 2) if j < 8 else (1, 3)
                        jj = j % 8
                        mv_chunk(hp_ctx, 2 * jj, pair)
                        mv_chunk(hp_ctx, 2 * jj + 1, pair)
                if hp_ctx is not None:
                    g_tail(hp_ctx)
                recip_az(h, 1)
                f_smalls(h, fresh, capture=capture, boost=boost)
                return {"half": h, "snap": snap, "capture": capture,
                        "boost": boost, "slots": None}

            def flush(ctx):
                """Standalone matvec burst + g_tail for the last half."""
                ctx["slots"] = [psU.tile([16, 512], F32, tag=f"slotp{i}",
                                         name=f"slotp{i}") for i in range(2)]
                for pair in ((0, 2), (1, 3)):
                    for nl in range(16):
                        mv_chunk(ctx, nl, pair)
                g_tail(ctx)

            # ---- iteration 0: fresh f + old-style fresh g (dynamic range) ----
            for half in range(2):
                for n in range(half * 16, (half + 1) * 16):
                    f_token(n, fresh=True)
                f_smalls(half, fresh=True, alpha_dst=rhs[1])
            for half in range(2):
                for n in range(half * 16, (half + 1) * 16):
                    f_token(n, fresh=True, p=1)
                f_smalls(half, fresh=True, p=1, write_sig_limbs=False,
                         alpha_dst=rhs[0], assemble_alpha=True)

            # ---- iteration 1: fresh f + pipelined matvec g ----
            ctx = steady_half(0, None, fresh=True)
            ctx = steady_half(1, ctx, fresh=True)

            # reload side-1 buffers with unsorted-coordinate statics (early;
            # consumed only by the final pass)
            for gg in range(2):
                nc.sync.dma_start(out=lhsT[1][gg * 64 : gg * 64 + 12, :],
                                  in_=lhsT1o_e.ap()[gg * 12 : (gg + 1) * 12, :])
                nc.sync.dma_start(out=rhs[1][gg * 64 : gg * 64 + 12, :],
                                  in_=rhs1o_e.ap()[gg * 12 : (gg + 1) * 12, :])

            # ---- steady iterations 2..iters-2 ----
            n_steady = iters - 3
            n_peel = n_steady % 8
            for _ in range(n_peel):
                ctx = steady_half(0, ctx)
                ctx = steady_half(1, ctx)
            n_loop = n_steady - n_peel
            if n_loop > 0:
                with tc.For_i(0, n_loop, 8, hint_engines=(
                        mybir.EngineType.PE, mybir.EngineType.DVE,
                        mybir.EngineType.Activation, mybir.EngineType.Pool)):
                    for _ in range(8):
                        ctx = steady_half(0, ctx)
                        ctx = steady_half(1, ctx)
            # ---- last iteration: boosted updates + capture ----
            bc = 1.0 + boost_c
            ctx = steady_half(0, ctx, capture=True, boost=bc)
            ctx = steady_half(1, ctx, capture=True, boost=bc)
            flush(ctx)

            # ---- final pass: unsort sigma1/alpha1 limbs, accumulate P ----
            for n in range(NTOK):
                px = [dp.tile([128, 1024], BF16, tag="fin", name=f"px{h}",
                              bufs=8)
                      for h in range(2)]
                py = [dp.tile([128, 1024], BF16, tag="fin", name=f"py{h}",
                              bufs=8)
                      for h in range(2)]
                for t in range(4):
                    r0 = (n * 4 + t) * 128
                    nc.sync.dma_start(out=px[t // 2][:, (t % 2) * D : (t % 2 + 1) * D],
                                      in_=permx_e.ap()[r0 : r0 + 128, :])
                    nc.sync.dma_start(out=py[t // 2][:, (t % 2) * D : (t % 2 + 1) * D],
                                      in_=permy_e.ap()[r0 : r0 + 128, :])
                pot = ps.tile([128, 1024], F32, tag="mm", name="pot")
                po1 = pot[0:3, 0:D]
                po2 = pot[0:3, D : 2 * D]
                for t in range(4):
                    col = n * 4 + t
                    nc.tensor.matmul(po1, Lcat[0][:, 3 * col : 3 * col + 3],
                                     px[t // 2][:, (t % 2) * D : (t % 2 + 1) * D],
                                     start=(t == 0), stop=(t == 3))
                    nc.tensor.matmul(po2, Lcat[1][:, 3 * col : 3 * col + 3],
                                     py[t // 2][:, (t % 2) * D : (t % 2 + 1) * D],
                                     start=(t == 0), stop=(t == 3))
                gg, bb = n % 2, n // 2
                stg = wk.tile([3, D], BF16, tag="stg", name="stg")
                with nc.allow_low_precision(reason="bf16 limb stage"):
                    nc.vector.tensor_copy(stg[:], po1)
                    nc.vector.tensor_copy(
                        rhs[1][gg * 64 : gg * 64 + 3, bb * 512 : (bb + 1) * 512], po2)
                nc.sync.dma_start(
                    out=lhsT[1][gg * 64 + 9 : gg * 64 + 12, bb * 512 : (bb + 1) * 512],
                    in_=stg[:])

                # final P accumulation for this token, full width, original
                # coordinates (interleaved so ACT exp overlaps PE unsort)
                for hh in range(2):
                    pt = ps.tile([128, 1024], F32, tag="mm", name="ptf")
                    for t in (2 * hh, 2 * hh + 1):
                        nc.tensor.matmul(
                            pt[:, (t % 2) * 512 : (t % 2 + 1) * 512],
                            lhsT[1][gg * 64 : gg * 64 + 12,
                                    bb * 512 + t * 128 : bb * 512 + (t + 1) * 128],
                            rhs[1][gg * 64 : gg * 64 + 12,
                                   bb * 512 : (bb + 1) * 512],
                            start=True, stop=True)
                    et = dp.tile([128, 1024], BF16, tag="fin", name="et",
                                 bufs=8)
                    nc.scalar.activation(et[:], pt[:], mybir.ActivationFunctionType.Exp,
                                         bias=la_bias[:], scale=1.0 / REG)
                    eng = pool_eng() if (n % 4 == 0) else nc.vector
                    eng.tensor_tensor(Pacc[:, hh * 1024 : (hh + 1) * 1024],
                                      Pacc[:, hh * 1024 : (hh + 1) * 1024],
                                      et[:], mybir.AluOpType.add)

            # AllReduce the P-sum across the 8 cores
            ccin = dr.tile([D, D], F32)
            ccout = dr.tile([D, D], F32, addr_space="Shared")
            for t in range(4):
                nc.sync.dma_start(out=ccin[:][t * 128 : (t + 1) * 128, :],
                                  in_=Pacc[:, t * D : (t + 1) * D])
            nc.gpsimd.collective_compute(
                "AllReduce", mybir.AluOpType.add,
                replica_groups=[list(range(NCORES))],
                ins=[ccin[:].opt()], outs=[ccout[:].opt()])
            for t in range(4):
                nc.sync.dma_start(out=Pacc[:, t * D : (t + 1) * D],
                                  in_=ccout[:][t * 128 : (t + 1) * 128, :])
            for t in range(4):
                dtile = wk.tile([128, D], F32, tag="dtile", name="dtile", bufs=1)
                nc.sync.dma_start(out=dtile[:],
                                  in_=delta_e.ap()[t * 128 : (t + 1) * 128, :])
                nc.vector.scalar_tensor_tensor(
                    out=Pacc[:, t * D : (t + 1) * D],
                    in0=Pacc[:, t * D : (t + 1) * D],
                    scalar=float(D * SCALE / NTOT), in1=dtile[:],
                    op0=mybir.AluOpType.mult, op1=mybir.AluOpType.add)
            po = ps.tile([128, 1024], F32, tag="mm", name="po")
            for t in range(4):
                nc.tensor.matmul(
                    po[0:NTOK, 0:D],
                    srcT[:, t * NTOK : (t + 1) * NTOK],
                    Pacc[:, t * D : (t + 1) * D],
                    start=(t == 0), stop=(t == 3))
            nc.vector.tensor_copy(out_sb[:], po[0:NTOK, 0:D])
            nc.sync.dma_start(out=out_e.ap(), in_=out_sb[:])

    nc.compile()
    return nc


def _repack(a):
    """[12, NTOK*512] -> [24, 8192]: token n -> partition group n%2, col block n//2."""
    return np.ascontiguousarray(
        a.reshape(12, 16, 2, 512).transpose(2, 0, 1, 3).reshape(24, 8192))


def _host_inputs(X, Y, delta_ot):
    src = np.ascontiguousarray(X.reshape(-1, D).astype(np.float32))
    tgt = np.ascontiguousarray(Y.reshape(-1, D).astype(np.float32))
    delta = np.ascontiguousarray(delta_ot.astype(np.float32))
    maps = []
    for c in range(NCORES):
        x = src[c * NTOK : (c + 1) * NTOK]
        y = tgt[c * NTOK : (c + 1) * NTOK]
        xi = np.argsort(x, axis=1)
        yi = np.argsort(y, axis=1)
        xs = np.take_along_axis(x, xi, axis=1)
        ys = np.take_along_axis(y, yi, axis=1)
        permx = np.zeros((NTOK, D, D), ml_dtypes.bfloat16)
        permy = np.zeros((NTOK, D, D), ml_dtypes.bfloat16)
        rows = np.arange(D)
        for n in range(NTOK):
            permx[n, rows, xi[n]] = 1
            permy[n, rows, yi[n]] = 1
        maps.append({
            "lhsT1": _repack(_lhsT_host(xs)).view(np.uint16),
            "lhsT2": _repack(_lhsT_host(ys)).view(np.uint16),
            "rhs1i": _repack(_rhs_host(-SCALE * ys * ys, 600.0 * ys)).view(np.uint16),
            "rhs2i": _repack(_rhs_host(np.zeros_like(xs), 600.0 * xs)).view(np.uint16),
            "lhsT1o": _repack(_lhsT_host(x)).view(np.uint16),
            "rhs1o": _repack(_rhs_host(np.zeros_like(y), 600.0 * y)).view(np.uint16),
            "permx": np.ascontiguousarray(permx.reshape(NTOK * D, D)).view(np.uint16),
            "permy": np.ascontiguousarray(permy.reshape(NTOK * D, D)).view(np.uint16),
            "xT": np.ascontiguousarray(x.T),
            "delta": delta,
        })
    return maps


_cache = {}


def _get_nc(iters=ITERS, margin=MARGIN, boost_c=BOOST_C):
    key = (iters, margin, boost_c)
    if key not in _cache:
        _cache[key] = _build(iters, margin, boost_c)
    return _cache[key]


def kernel(X, Y, delta_ot, _iters=ITERS, _margin=MARGIN, _boost=BOOST_C,
           _trace=False):
    nc = _get_nc(_iters, _margin, _boost)
    maps = _host_inputs(np.asarray(X), np.asarray(Y), np.asarray(delta_ot))
    res = run_bass_kernel_spmd(nc, maps, list(range(NCORES)), trace=_trace)
    out = np.concatenate([res.results[c]["out"] for c in range(NCORES)], axis=0)
    B, S = 2, 128
    out = out.reshape(B, S, D).astype(np.float32)
    if _trace:
        return out, res
    return out
